# revision 43
# baseline (speedup 1.0000x reference)
"""Multi-head attention (softmax over the HEADS axis) on 8 trn2 NeuronCores.

Reference math (B=2, S=2048, D=512, H=8, Dk=64):
    q = split_heads(Q @ w_q.T + b_q)          # [B,H,S,Dk]
    scores = q @ k.T / sqrt(Dk)               # [B,H,Sq,Sk]
    probs = softmax(scores, axis=1)           # softmax over H (source quirk!)
    attn = probs @ v                          # [B,H,Sq,Dk]
    out = concat_heads(attn) @ w_o.T + b_o    # [B,S,D]

Because softmax is over H, it is local to each (b, sq, sk) position: sharding
over (batch x query rows) needs no cross-core communication.  Core c handles
batch c//4, query rows (c%4)*512 .. +512, with all 8 heads resident.

All matmul operands are bf16 (fp32 matmul runs LOW_HIGH = 2x instructions);
PSUM accumulation stays fp32.  Steady state is elementwise-bound: per kj tile
the budget is ~4 us on each of ACT (exp), DVE and GpSimd (head-sum tree,
reciprocal, normalize), so that work is split across all three.

Layouts (contraction dim always on SBUF partitions):
  qt/kt/vt  [128, 4, S*]  = X.T      (din = chunk*128 + p), bf16
  w*t       [128, 4, 512] = W.T      (din = chunk*128 + p), bf16
  qTs/kTs   [128, 4, S*]  = proj.T   (dout = m*128 + p), bf16
  vs        [128, 16, 512] = v natural (kj on partitions), bf16
  scores    psum [kj=128, 2, 512qi] per head pair -> exp -> softmax over h
  attn      psum [2*64=128 d, 512 qi] per head-pair, accumulated over kj tiles
  out       [qi, 512] natural, fp32
"""

import numpy as np

B, S, D, H, DK = 2, 2048, 512, 8, 64
NCORES = 8
CPB = NCORES // B          # cores per batch
QI = S // CPB              # query rows per core (512)
KJT = 128                  # kj tile (partition dim of scores)
NKJ = S // KJT             # 16 kj tiles
NC_, CH = 128, 4           # partitions, din chunks
SCALE = 1.0 / np.sqrt(DK)  # folded into exp activation


def _chunk(x, dt):
    """[512, F] -> [128, 4, F] with row = chunk*128 + p."""
    f = x.shape[1]
    return np.ascontiguousarray(
        np.ascontiguousarray(x).reshape(CH, NC_, f).transpose(1, 0, 2)
    ).astype(dt)


def _build(with_bias):
    from contextlib import ExitStack

    import concourse.bass as bass
    import concourse.mybir as mybir
    import concourse.tile as tile
    from concourse import bacc

    fp32 = mybir.dt.float32
    bf16 = mybir.dt.bfloat16

    nc = bacc.Bacc(
        "TRN2",
        target_bir_lowering=False,
        debug=False,
        enable_asserts=False,
        num_devices=NCORES,
    )

    def din(name, shape):
        return nc.dram_tensor(name, shape, bf16, kind="ExternalInput").ap()

    qt_d = din("qt", [NC_, CH, QI])
    kt_d = din("kt", [NC_, CH, S])
    vt_d = din("vt", [NC_, CH, S])
    w_d = {n: din(n, [NC_, CH, D]) for n in ("wqt", "wkt", "wvt", "wot")}
    if with_bias:
        b_d = {n: din(n, [1, D]) for n in ("bq", "bk", "bv", "bo")}
    out_d = nc.dram_tensor("out", [QI, D], fp32, kind="ExternalOutput").ap()

    with tile.TileContext(nc) as tc, ExitStack() as ctx:
        wpool = ctx.enter_context(tc.tile_pool(name="wts", bufs=2))
        raw = ctx.enter_context(tc.tile_pool(name="raw", bufs=5))
        acts = ctx.enter_context(tc.tile_pool(name="acts", bufs=1))
        sm = ctx.enter_context(tc.tile_pool(name="sm", bufs=4))
        pp = ctx.enter_context(tc.tile_pool(name="pp", bufs=8))
        ps = ctx.enter_context(tc.tile_pool(name="ps", bufs=2, space="PSUM"))
        psa = ctx.enter_context(tc.tile_pool(name="psa", bufs=4, space="PSUM"))

        qTs = acts.tile([NC_, CH, QI], bf16, tag="qTs")
        kTs = acts.tile([NC_, CH, S], bf16, tag="kTs")
        vs = acts.tile([NC_, NKJ, D], bf16, tag="vs")
        attnT = acts.tile([NC_, CH, QI], bf16, tag="attnT")
        outsb = acts.tile([NC_, CH, D], fp32, tag="outsb")

        if with_bias:
            ones = acts.tile([1, D], bf16, tag="ones")
            nc.vector.memset(ones, 1.0)
            brow = {}
            for n in ("bq", "bk", "bv", "bo"):
                brow[n] = acts.tile([1, D], bf16, tag=n, name=n)
                nc.sync.dma_start(out=brow[n], in_=b_d[n])

        wsb = {}
        weng = [nc.sync, nc.sync, nc.sync, nc.sync]
        for ei, n in enumerate(("wqt", "wkt", "wvt", "wot")):
            wsb[n] = wpool.tile([NC_, CH, D], bf16, tag="w", name=n)
            weng[ei].dma_start(out=wsb[n], in_=w_d[n])

        def bias_mm(pt_ap, bname, col_slice):
            """rank-1 bias init: psum = bias-row (x) ones-row (or flipped)."""
            if col_slice is not None:  # bias along partitions
                lhsT = brow[bname][:, col_slice]
                rhs = ones[:, : pt_ap.shape[-1]]
            else:  # bias along free dim
                lhsT = ones[:, :128]
                rhs = brow[bname]
            nc.tensor.matmul(pt_ap, lhsT=lhsT, rhs=rhs, start=True, stop=False)

        # ---------------- projections ----------------
        # Q: qT[dout, qi] = wqT[din,dout].T @ QT[din, qi]
        qraw = raw.tile([NC_, CH, QI], bf16, tag="raw")
        nc.sync.dma_start(out=qraw, in_=qt_d)
        for m in range(4):
            pt = psa.tile([NC_, 1, 512], fp32, tag="attn")
            if with_bias:
                bias_mm(pt[:, 0, :QI], "bq", slice(m * 128, (m + 1) * 128))
            for c in range(CH):
                nc.tensor.matmul(
                    pt[:, 0, :QI],
                    lhsT=wsb["wqt"][:, c, m * 128 : (m + 1) * 128],
                    rhs=qraw[:, c, :],
                    start=(c == 0 and not with_bias),
                    stop=(c == CH - 1),
                )
            if m % 2 == 0:
                nc.scalar.copy(qTs[:, m, :], pt[:, 0, :QI])
            else:
                nc.vector.tensor_copy(qTs[:, m, :], pt[:, 0, :QI])

        # K: kT[dout, kj]
        kraw = [raw.tile([NC_, S], bf16, tag="raw", name=f"kraw{c}") for c in range(CH)]
        for c in range(CH):
            weng[c].dma_start(out=kraw[c], in_=kt_d[:, c, :])
        for m in range(4):
            for kc in range(4):
                pt = psa.tile([NC_, 1, 512], fp32, tag="attn")
                if with_bias:
                    bias_mm(pt[:, 0, :], "bk", slice(m * 128, (m + 1) * 128))
                for c in range(CH):
                    nc.tensor.matmul(
                        pt[:, 0, :],
                        lhsT=wsb["wkt"][:, c, m * 128 : (m + 1) * 128],
                        rhs=kraw[c][:, kc * 512 : (kc + 1) * 512],
                        start=(c == 0 and not with_bias),
                        stop=(c == CH - 1),
                    )
                if kc % 2 == 0:
                    nc.scalar.copy(
                        kTs[:, m, kc * 512 : (kc + 1) * 512], pt[:, 0, :]
                    )
                else:
                    nc.vector.tensor_copy(
                        kTs[:, m, kc * 512 : (kc + 1) * 512], pt[:, 0, :]
                    )

        # V: v natural [kj, dout] = VT[din,kj].T @ wvT[din,dout]
        vraw = [raw.tile([NC_, S], bf16, tag="raw", name=f"vraw{c}") for c in range(CH)]
        for c in range(CH):
            weng[c].dma_start(out=vraw[c], in_=vt_d[:, c, :])
        for t in range(NKJ):
            pt = psa.tile([NC_, 1, 512], fp32, tag="attn")
            if with_bias:
                bias_mm(pt[:, 0, :], "bv", None)
            for c in range(CH):
                nc.tensor.matmul(
                    pt[:, 0, :],
                    lhsT=vraw[c][:, t * 128 : (t + 1) * 128],
                    rhs=wsb["wvt"][:, c, :],
                    start=(c == 0 and not with_bias),
                    stop=(c == CH - 1),
                )
            nc.scalar.copy(vs[:, t, :], pt[:, 0, :])

        # ---------------- attention ----------------
        # attn psum: tile dc holds heads 2dc (p 0..63), 2dc+1 (p 64..127)
        at = [psa.tile([NC_, 512], fp32, tag="attn", name=f"at{i}") for i in range(4)]

        def emit_attn(td, prs):
            for h in range(H):
                po = (h % 2) * 64
                nc.tensor.matmul(
                    at[h // 2][po : po + 64, :QI],
                    lhsT=vs[:, td, h * 64 : (h + 1) * 64],
                    rhs=prs[h // 4][:, h % 4, :],
                    start=(td == 0),
                    stop=(td == NKJ - 1),
                )

        LAG = 3
        pending = []
        for t in range(NKJ):
            exp_t = sm.tile([NC_, H, QI], bf16, tag="exp", bufs=6)
            for m in range(4):
                spt = ps.tile([NC_, 2, 512], fp32, tag="ps")
                for j in range(2):
                    po = j * 64
                    nc.tensor.matmul(
                        spt[:, j, :QI],
                        lhsT=kTs[po : po + 64, m, t * 128 : (t + 1) * 128],
                        rhs=qTs[po : po + 64, m, :],
                        start=True,
                        stop=True,
                    )
                nc.scalar.activation(
                    exp_t[:, 2 * m : 2 * m + 2, :],
                    spt[:, :, :],
                    mybir.ActivationFunctionType.Exp,
                    scale=SCALE,
                )

            # head-sum tree, split across gpsimd (slow) and DVE to balance:
            # gpsimd: L1a full + L1b first half; DVE: L1b second half, L2, L3
            s01 = sm.tile([NC_, 2, QI], bf16, tag="s01")
            s23 = sm.tile([NC_, 2, QI], bf16, tag="s23")
            nc.gpsimd.tensor_add(s01, exp_t[:, 0:2, :], exp_t[:, 2:4, :])
            nc.gpsimd.tensor_add(s23, exp_t[:, 4:6, :], exp_t[:, 6:8, :])
            nc.vector.tensor_add(s01, s01, s23)
            ssum = sm.tile([NC_, QI], fp32, tag="ssum")
            nc.vector.tensor_add(ssum, s01[:, 0, :], s01[:, 1, :])
            # fast reciprocal writing bf16 directly (DVE converts on the
            # final write; the fp32 bit-trick only needs the fp32 *input*)
            from concourse.dve_ops import (
                RECIP_APPROX_FAST_CONSTS as _RC,
                RECIPROCAL_APPROX_FAST as _RF,
            )
            r = sm.tile([NC_, QI], bf16, tag="r")
            nc.vector._custom_dve(
                _RF, out=r, in0=ssum, s0=_RC["s0"], s1=_RC["s1"], imm2=_RC["imm2"]
            )

            # normalize per head: plain contiguous operands keep DVE 2x mode;
            # one head's mul goes to gpsimd to shave the DVE stream
            prs = []
            for g in range(2):
                pr = pp.tile([NC_, 4, QI], bf16, tag="probs")
                for jj in range(4):
                    nc.vector.tensor_mul(
                        pr[:, jj, :], exp_t[:, 4 * g + jj, :], r
                    )
                prs.append(pr)

            # attn matmuls run LAG tiles behind (probs already ready -> PE
            # never stalls mid-stream on the softmax chain)
            pending.append((t, prs))
            if len(pending) > LAG:
                emit_attn(*pending.pop(0))

        for td, prs in pending:
            emit_attn(td, prs)

        for dc in range(4):
            eng = nc.vector if dc % 2 == 0 else nc.scalar
            if dc % 2 == 0:
                nc.vector.tensor_copy(attnT[:, dc, :], at[dc][:, :QI])
            else:
                nc.scalar.copy(attnT[:, dc, :], at[dc][:, :QI])

        # ---------------- output projection ----------------
        for m in range(4):
            ot = psa.tile([NC_, 512], fp32, tag="attn")
            if with_bias:
                bias_mm(ot, "bo", None)
            for c in range(CH):
                nc.tensor.matmul(
                    ot,
                    lhsT=attnT[:, c, m * 128 : (m + 1) * 128],
                    rhs=wsb["wot"][:, c, :],
                    start=(c == 0 and not with_bias),
                    stop=(c == CH - 1),
                )
            if m % 2 == 0:
                nc.scalar.copy(outsb[:, m, :], ot)
            else:
                nc.vector.tensor_copy(outsb[:, m, :], ot)
            nc.sync.dma_start(
                out=out_d.rearrange("(m p) o -> p m o", p=NC_)[:, m, :],
                in_=outsb[:, m, :],
            )

    nc.compile()
    return nc


_CACHE = {}


def kernel(Q, K, V, w_q, b_q, w_k, b_k, w_v, b_v, w_o, b_o, _trace=False):
    import ml_dtypes
    from concourse import bass_utils

    bf = ml_dtypes.bfloat16
    Q = np.asarray(Q, np.float32)
    K = np.asarray(K, np.float32)
    V = np.asarray(V, np.float32)
    with_bias = any(
        np.any(np.asarray(b) != 0) for b in (b_q, b_k, b_v, b_o)
    )

    if ("nc", with_bias) not in _CACHE:
        _CACHE[("nc", with_bias)] = _build(with_bias)
    nc = _CACHE[("nc", with_bias)]

    wmaps = {
        "wqt": _chunk(np.asarray(w_q, np.float32).T, bf),
        "wkt": _chunk(np.asarray(w_k, np.float32).T, bf),
        "wvt": _chunk(np.asarray(w_v, np.float32).T, bf),
        "wot": _chunk(np.asarray(w_o, np.float32).T, bf),
    }
    if with_bias:
        for n, b in (("bq", b_q), ("bk", b_k), ("bv", b_v), ("bo", b_o)):
            wmaps[n] = np.ascontiguousarray(
                np.asarray(b, np.float32).reshape(1, D)
            ).astype(bf)

    in_maps = []
    for c in range(NCORES):
        b = c // CPB
        s0 = (c % CPB) * QI
        in_maps.append(
            dict(
                wmaps,
                qt=_chunk(Q[b, s0 : s0 + QI, :].T, bf),
                kt=_chunk(K[b].T, bf),
                vt=_chunk(V[b].T, bf),
            )
        )

    res = bass_utils.run_bass_kernel_spmd(
        nc, in_maps, core_ids=list(range(NCORES)), trace=_trace
    )

    out = np.empty((B, S, D), np.float32)
    for c in range(NCORES):
        b = c // CPB
        s0 = (c % CPB) * QI
        out[b, s0 : s0 + QI, :] = res.results[c]["out"]
    if _trace:
        kernel._last_results = res
    return out
